# revision 14
# baseline (speedup 1.0000x reference)
"""BeatFCOS (1D FCOS detection head) Trainium2 Bass kernel.

Strategy: data-parallel over batch (8 samples -> 8 NeuronCores, SPMD).
Each core runs the full FPN + cls/reg heads for one sample.

  - convs as f32r matmuls (PSUM fp32 accumulate), 512-wide L tiles
  - GroupNorm: bn_stats/bn_aggr per channel, conv-bias folded into GN
    coefficients, cross-partition group pooling via tiny blockdiag matmuls
  - GN apply + ReLU fused into one ScalarE activation per tile
  - P4 feature map (L=8192) spilled to DRAM and streamed for head conv1
  - outputs written as [5, 15872] per core ((cls0,cls1,reg0,reg1,left) x L),
    transposed/split on host

Self-contained: hardcodes all shapes from the problem spec.
"""
import sys
sys.path.insert(0, "/opt/trn_rl_repo")
import numpy as np

import concourse.bass as bass
import concourse.bacc as bacc
import concourse.tile as tile
from concourse import mybir
from concourse.bass_utils import run_bass_kernel_spmd

F32 = mybir.dt.float32
F32R = mybir.dt.float32r
AF = mybir.ActivationFunctionType
ALU = mybir.AluOpType

B = 8
FEAT = 256
GROUPS = 32
EPS = 1e-5
LT = 512                      # L-tile width (= one PSUM bank of fp32)
L4, L5, L6, L7, L8 = 8192, 4096, 2048, 1024, 512
LTOT = L4 + L5 + L6 + L7 + L8          # 15872
OUT_OFF = {"P4": 0, "P5": L4, "P6": L4 + L5, "P7": L4 + L5 + L6,
           "P8": L4 + L5 + L6 + L7}

# ---------------------------------------------------------------- host packing


def _lhsT_pack(w):
    """w: [Cout, Cin, K] -> [n_tiles, 128, M] ordered (cib, t, cob), lhsT[ci, co]."""
    Cout, Cin, K = w.shape
    cibs, cobs = Cin // 128, max(Cout // 128, 1)
    M = 128 if Cout >= 128 else Cout
    arr = np.ascontiguousarray(np.transpose(w, (1, 2, 0)))  # [Cin, K, Cout]
    tiles = []
    for cib in range(cibs):
        for t in range(K):
            for cob in range(cobs):
                tiles.append(arr[cib * 128:(cib + 1) * 128, t,
                                 cob * M:(cob + 1) * M])
    return np.ascontiguousarray(np.stack(tiles)).astype(np.float32)


def _col2(v):
    """[256] -> [128, 2] (col j = cob j)."""
    return np.ascontiguousarray(np.asarray(v, np.float32).reshape(2, 128).T)


def _prep_params(fpn_p, cls_p, reg_p):
    g = lambda x: np.asarray(x, np.float32)
    weights = {
        "w_p5_1": _lhsT_pack(g(fpn_p["p5_1"][0])),
        "w_p5_2": _lhsT_pack(g(fpn_p["p5_2"][0])),
        "w_p4_1": _lhsT_pack(g(fpn_p["p4_1"][0])),
        "w_p4_2": _lhsT_pack(g(fpn_p["p4_2"][0])),
        "w_p6": _lhsT_pack(g(fpn_p["p6"][0])),
        "w_p7": _lhsT_pack(g(fpn_p["p7_2"][0])),
        "w_p8": _lhsT_pack(g(fpn_p["p8_2"][0])),
        "w_cls1": _lhsT_pack(g(cls_p["conv1"][0])),
        "w_cls2": _lhsT_pack(g(cls_p["conv2"][0])),
        "w_reg1": _lhsT_pack(g(reg_p["conv1"][0])),
        "w_reg2": _lhsT_pack(g(reg_p["conv2"][0])),
        "w_clso": _lhsT_pack(g(cls_p["out"][0])),                # [6,128,2]
        "w_rlo": _lhsT_pack(np.concatenate(
            [g(reg_p["reg"][0]), g(reg_p["left"][0])], axis=0)),  # [6,128,3]
    }
    # vecs: [128, NV] fp32, column map below
    cols = []
    VC = {}

    def add2(name, arr128x2):
        VC[name] = len(cols)
        cols.append(arr128x2[:, 0])
        cols.append(arr128x2[:, 1])

    def add1(name, vec, rows):
        VC[name] = len(cols)
        c = np.zeros(128, np.float32)
        c[:rows] = vec
        cols.append(c)

    for nm, p in [("p5_1", fpn_p["p5_1"]), ("p5_2", fpn_p["p5_2"]),
                  ("p4_1", fpn_p["p4_1"]), ("p4_2", fpn_p["p4_2"]),
                  ("p6", fpn_p["p6"]), ("p7", fpn_p["p7_2"]),
                  ("p8", fpn_p["p8_2"])]:
        add2("b_" + nm, _col2(p[1]))
    for nm, p in [("cls1", cls_p["conv1"]), ("cls2", cls_p["conv2"]),
                  ("reg1", reg_p["conv1"]), ("reg2", reg_p["conv2"])]:
        add2("b_" + nm, _col2(p[1]))
    add2("g_cls1", _col2(cls_p["gn1"][0])); add2("be_cls1", _col2(cls_p["gn1"][1]))
    add2("g_cls2", _col2(cls_p["gn2"][0])); add2("be_cls2", _col2(cls_p["gn2"][1]))
    add2("g_reg1", _col2(reg_p["gn1"][0])); add2("be_reg1", _col2(reg_p["gn1"][1]))
    add2("g_reg2", _col2(reg_p["gn2"][0])); add2("be_reg2", _col2(reg_p["gn2"][1]))
    add1("eps", np.full(128, EPS, np.float32), 128)
    add1("b_clso", g(cls_p["out"][1]), 2)
    add1("b_rlo", np.concatenate([g(reg_p["reg"][1]), g(reg_p["left"][1])]), 3)
    vecs = np.ascontiguousarray(np.stack(cols, axis=1))
    VC["__ncols"] = vecs.shape[1]

    p2g = np.zeros((128, 16), np.float32)
    for gi in range(16):
        p2g[gi * 8:(gi + 1) * 8, gi] = 1.0 / 8.0
    g2p = np.zeros((16, 128), np.float32)
    for gi in range(16):
        g2p[gi, gi * 8:(gi + 1) * 8] = 1.0
    weights["vecs"] = vecs
    weights["p2g"] = p2g
    weights["g2p"] = g2p
    return weights, VC


# ---------------------------------------------------------------- device build

class KB:
    """Kernel builder state."""

    def __init__(self, nc, tc, VC):
        self.nc, self.tc, self.VC = nc, tc, VC
        self.pools = {}
        self.wdram = {}
        self.wsb_cache = {}

    def vcol(self, name, n=1, rows=128):
        c = self.VC[name]
        return self.vecs_sb[0:rows, c:c + n]

    def get_w(self, name):
        """Stream a conv-weight pack into SBUF (tag-shared double buffer)."""
        if name in self.wsb_cache:
            return self.wsb_cache[name]
        d = self.wdram[name]
        n, _, m = d.shape
        t = self.pools["wt"].tile([128, n * m], F32R, tag="wt")
        self.nc.sync.dma_start(out=t.rearrange("p (n m) -> p n m", n=n),
                               in_=d.rearrange("n p m -> p n m"))
        return t.rearrange("p (n m) -> p n m", n=n)

    def keep_w(self, name):
        """Load a weight pack into a resident (bufs=1) slot."""
        d = self.wdram[name]
        n, _, m = d.shape
        t = self.pools["const"].tile([128, n * m], F32R, tag="w_" + name)
        self.nc.sync.dma_start(out=t.rearrange("p (n m) -> p n m", n=n),
                               in_=d.rearrange("n p m -> p n m"))
        self.wsb_cache[name] = t.rearrange("p (n m) -> p n m", n=n)
        return self.wsb_cache[name]


def _strided(ap_slice, step, count):
    return bass.AP(tensor=ap_slice.tensor, offset=ap_slice.offset,
                   ap=[ap_slice.ap[0], [step, count]])


def alloc_fmap(kb, pool, L, tag):
    """Allocate a padded [2][128, L+2] f32r fmap, zero the pad columns."""
    nc = kb.nc
    ts = []
    for cob in range(2):
        t = pool.tile([128, L + 2], F32R, tag=tag)
        nc.vector.tensor_copy(t[:, 0:1], kb.zt32)
        nc.vector.tensor_copy(t[:, L + 1:L + 2], kb.zt32)
        ts.append(t)
    return ts


def conv256(kb, w_sb, rhs_fn, n_cib, out_cb, L, stats=None, psname="ps"):
    """Generic Cin=(n_cib*128) -> 256 conv over L outputs (pre-sliced rhs via
    rhs_fn(cib, t, lt) -> AP [128, 512]). out_cb(cob, lt, psum) emits epilogue.
    Weight pack w_sb: [128, n, M] ordered (cib, t, cob). taps inferred."""
    nc = kb.nc
    n = w_sb.shape[1]
    taps = n // (n_cib * 2)
    for lt in range(L // LT):
        for cob in range(2):
            ps = kb.pools["psum"].tile([128, LT], F32, tag=psname, bufs=3)
            k = 0
            for cib in range(n_cib):
                for t in range(taps):
                    idx = (cib * taps + t) * 2 + cob
                    nc.tensor.matmul(ps, lhsT=w_sb[:, idx, :],
                                     rhs=rhs_fn(cib, t, lt),
                                     start=(k == 0), stop=(k == n_cib * taps - 1))
                    k += 1
            out_cb(cob, lt, ps)


def emit_gn_coefs(kb, stats_pair, nch, bc2, gamma2, beta2):
    """stats_pair: 2 tiles [128, nch, 6]. Returns (scale2, bias2) [128,2] fp32."""
    nc = kb.nc
    coef = kb.pools["coef"]
    mv = coef.tile([128, 2, 2], F32, tag="mv")
    for cob in range(2):
        nc.vector.bn_aggr(out=mv[:, cob, :], in_=stats_pair[cob])
    mean_v = mv[:, :, 0]          # [128, 2] strided
    var_v = mv[:, :, 1]
    pm = coef.tile([128, 4], F32R, tag="pm")
    s1 = coef.tile([128, 2], F32, tag="s1")
    s2 = coef.tile([128, 2], F32, tag="s2")
    s3 = coef.tile([128, 2], F32, tag="s3")
    nc.vector.tensor_tensor(out=pm[:, 0:2], in0=mean_v, in1=bc2, op=ALU.add)
    nc.vector.tensor_tensor(out=s1, in0=mean_v, in1=mean_v, op=ALU.mult)
    nc.vector.tensor_tensor(out=s1, in0=s1, in1=var_v, op=ALU.add)      # m2
    nc.vector.tensor_scalar(out=s2, in0=mean_v, scalar1=2.0, scalar2=None,
                            op0=ALU.mult)
    nc.vector.tensor_tensor(out=s2, in0=s2, in1=bc2, op=ALU.add)
    nc.vector.tensor_tensor(out=s2, in0=s2, in1=bc2, op=ALU.mult)
    nc.vector.tensor_tensor(out=pm[:, 2:4], in0=s1, in1=s2, op=ALU.add)  # m2b
    gps = kb.pools["psum"].tile([16, 4], F32, tag="gps")
    nc.tensor.matmul(gps, lhsT=kb.p2g_sb, rhs=pm, start=True, stop=True)
    gsb = coef.tile([16, 4], F32R, tag="gsb")
    nc.vector.tensor_copy(gsb, gps)
    bps = kb.pools["psum"].tile([128, 4], F32, tag="bps")
    nc.tensor.matmul(bps, lhsT=kb.g2p_sb, rhs=gsb, start=True, stop=True)
    bcp = coef.tile([128, 4], F32, tag="bcp")
    nc.vector.tensor_copy(bcp, bps)
    # var_g = m2g - mean_g^2 ; rstd = 1/sqrt(var+eps)
    nc.vector.tensor_tensor(out=s1, in0=bcp[:, 0:2], in1=bcp[:, 0:2], op=ALU.mult)
    nc.vector.tensor_tensor(out=s1, in0=bcp[:, 2:4], in1=s1, op=ALU.subtract)
    nc.scalar.activation(out=s2, in_=s1, func=AF.Sqrt,
                         bias=kb.vcol("eps"), scale=1.0)
    nc.vector.reciprocal(out=s2, in_=s2)                                 # rstd
    scale2 = coef.tile([128, 2], F32, tag="scale2")
    bias2 = coef.tile([128, 2], F32, tag="bias2")
    nc.vector.tensor_tensor(out=scale2, in0=gamma2, in1=s2, op=ALU.mult)
    nc.vector.tensor_tensor(out=s3, in0=bc2, in1=bcp[:, 0:2], op=ALU.subtract)
    nc.vector.tensor_tensor(out=s3, in0=s3, in1=scale2, op=ALU.mult)
    nc.vector.tensor_tensor(out=bias2, in0=s3, in1=beta2, op=ALU.add)
    return scale2, bias2


def gn_apply(kb, o_pair, L, scale2, bias2):
    nc = kb.nc
    for cob in range(2):
        for lt in range(L // LT):
            sl = o_pair[cob][:, 1 + lt * LT: 1 + (lt + 1) * LT]
            nc.scalar.activation(out=sl, in_=sl, func=AF.Relu,
                                 bias=bias2[:, cob:cob + 1],
                                 scale=scale2[:, cob:cob + 1])


def head_conv(kb, w_name, rhs_fn, n_cib, L, o_pool, otag, stats_tag):
    """conv + copy-out (f32r) + bn_stats. Returns (o_pair, stats_pair)."""
    nc = kb.nc
    o_pair = alloc_fmap(kb, o_pool, L, otag)
    nch = L // LT
    stats = [kb.pools["stats"].tile([128, nch, 6], F32, tag=stats_tag + str(c),
                                    name=f"stats_{stats_tag}{c}")
             for c in range(2)]
    w_sb = kb.get_w(w_name)

    def cb(cob, lt, ps):
        sl = o_pair[cob][:, 1 + lt * LT: 1 + (lt + 1) * LT]
        nc.vector.tensor_copy(sl, ps)
        nc.vector.bn_stats(out=stats[cob][:, lt, :], in_=sl)

    conv256(kb, w_sb, rhs_fn, n_cib, cb, L)
    return o_pair, stats


def rhs_resident(x_pair):
    """rhs provider for a resident padded fmap pair."""
    def fn(cib, t, lt):
        return x_pair[cib][:, lt * LT + t: lt * LT + t + LT]
    return fn


def out_convs(kb, o2n, which, L, out_d, off):
    """Final 1x3 convs for one head + epilogue + DMA to out rows."""
    nc = kb.nc
    for lt in range(L // LT):
        if which == "cls":
            cps = kb.pools["psum"].tile([2, LT], F32, tag="ps", bufs=3)
            for k, (cib, t) in enumerate(
                    [(c, t) for c in range(2) for t in range(3)]):
                nc.tensor.matmul(cps, lhsT=kb.wsb_cache["w_clso"][:, cib * 3 + t, :],
                                 rhs=o2n[cib][:, lt * LT + t: lt * LT + t + LT],
                                 start=(k == 0), stop=(k == 5))
            cst = kb.pools["st"].tile([2, LT], F32, tag="cst", bufs=2)
            nc.scalar.activation(out=cst, in_=cps, func=AF.Sigmoid,
                                 bias=kb.vcol("b_clso", rows=2), scale=1.0)
            nc.sync.dma_start(out=out_d[0:2, off + lt * LT: off + (lt + 1) * LT],
                              in_=cst)
        else:
            rps = kb.pools["psum"].tile([3, LT], F32, tag="ps", bufs=3)
            for k, (cib, t) in enumerate(
                    [(c, t) for c in range(2) for t in range(3)]):
                nc.tensor.matmul(rps, lhsT=kb.wsb_cache["w_rlo"][:, cib * 3 + t, :],
                                 rhs=o2n[cib][:, lt * LT + t: lt * LT + t + LT],
                                 start=(k == 0), stop=(k == 5))
            rst = kb.pools["st"].tile([3, LT], F32, tag="rst", bufs=2)
            # sigmoid all 3 rows (only row 2 = left is kept) ...
            nc.scalar.activation(out=rst, in_=rps, func=AF.Sigmoid,
                                 bias=kb.vcol("b_rlo", rows=3), scale=1.0)
            # ... then overwrite rows 0:2 with the plain reg bias-add
            nc.vector.tensor_scalar(out=rst[0:2, :], in0=rps[0:2, :],
                                    scalar1=kb.vcol("b_rlo", rows=2), scalar2=None,
                                    op0=ALU.add)
            nc.sync.dma_start(out=out_d[2:5, off + lt * LT: off + (lt + 1) * LT],
                              in_=rst)


def run_head(kb, kind, rhs1_fn, n_cib, L, o_pool):
    """One head (cls or reg): conv1 -> GN1 -> conv2 -> GN2. Returns o2n pair."""
    o1, st1 = head_conv(kb, f"w_{kind}1", rhs1_fn, n_cib, L, o_pool,
                        "o1", f"s1{kind}")
    s1c, b1c = emit_gn_coefs(kb, st1, L // LT, kb.vcol(f"b_{kind}1", 2),
                             kb.vcol(f"g_{kind}1", 2), kb.vcol(f"be_{kind}1", 2))
    gn_apply(kb, o1, L, s1c, b1c)
    o2, st2 = head_conv(kb, f"w_{kind}2", rhs_resident(o1), 2, L, o_pool,
                        "o2", f"s2{kind}")
    s2c, b2c = emit_gn_coefs(kb, st2, L // LT, kb.vcol(f"b_{kind}2", 2),
                             kb.vcol(f"g_{kind}2", 2), kb.vcol(f"be_{kind}2", 2))
    gn_apply(kb, o2, L, s2c, b2c)
    return o2


def rhs_stream(kb, fb, hint):
    """conv1 rhs provider streaming [128, LT+2] windows from a DRAM fmap."""
    nc = kb.nc
    cache = {}

    def fn(cib, t, lt):
        key = (cib, lt)
        if key not in cache:
            xk = kb.pools["xk"].tile([128, LT + 2], F32R, tag="xk",
                                     name=f"xk_{hint}")
            nc.sync.dma_start(out=xk,
                              in_=fb[cib][:, lt * LT: lt * LT + LT + 2])
            cache[key] = xk
        return cache[key][:, t: t + LT]
    return fn


def zero_dram_pads(kb, fb, L):
    nc = kb.nc
    for cib in range(2):
        nc.sync.dma_start(out=fb[cib][:, 0:1], in_=kb.zt)
        nc.sync.dma_start(out=fb[cib][:, L + 1:L + 2], in_=kb.zt)


def cb_stream(kb, fb, bcol, relu_pair=None):
    """Epilogue: bias-add -> DMA to DRAM fmap; optionally also ReLU -> SBUF pair."""
    nc = kb.nc

    def cb(cob, lt, ps):
        ts = kb.pools["st"].tile([128, LT], F32R, tag="ts", bufs=3, name="ts")
        nc.vector.tensor_scalar(out=ts, in0=ps, scalar1=bcol[:, cob:cob + 1],
                                scalar2=None, op0=ALU.add)
        nc.sync.dma_start(out=fb[cob][:, 1 + lt * LT: 1 + (lt + 1) * LT], in_=ts)
        if relu_pair is not None:
            nc.scalar.activation(
                out=relu_pair[cob][:, 1 + lt * LT: 1 + (lt + 1) * LT],
                in_=ps, func=AF.Relu, bias=bcol[:, cob:cob + 1], scale=1.0)
    return cb


def build():
    nc = bacc.Bacc()
    c4_d = nc.declare_dram_parameter("C4", [128, L4], F32R, isOutput=False)
    c5_d = nc.declare_dram_parameter("C5", [2, 128, L5], F32R, isOutput=False)
    wshapes = {
        "w_p5_1": [4, 128, 128], "w_p5_2": [12, 128, 128],
        "w_p4_1": [2, 128, 128], "w_p4_2": [12, 128, 128],
        "w_p6": [12, 128, 128], "w_p7": [12, 128, 128], "w_p8": [12, 128, 128],
        "w_cls1": [12, 128, 128], "w_cls2": [12, 128, 128],
        "w_reg1": [12, 128, 128], "w_reg2": [12, 128, 128],
        "w_clso": [6, 128, 2], "w_rlo": [6, 128, 3],
    }
    _, VC = _prep_params(*_dummy_params())
    NV = VC.pop("__ncols")
    vecs_d = nc.declare_dram_parameter("vecs", [128, NV], F32, isOutput=False)
    p2g_d = nc.declare_dram_parameter("p2g", [128, 16], F32R, isOutput=False)
    g2p_d = nc.declare_dram_parameter("g2p", [16, 128], F32R, isOutput=False)
    out_d = nc.declare_dram_parameter("out", [5, LTOT], F32, isOutput=True)
    LVL = [("P4", L4), ("P5", L5), ("P6", L6), ("P7", L7), ("P8", L8)]
    fbs = {nm: nc.dram_tensor(f"fb_{nm}", [2, 128, L + 2], F32R)
           for nm, L in LVL}

    with tile.TileContext(nc) as tc:
        kb = KB(nc, tc, VC)
        for nm, shp in wshapes.items():
            kb.wdram[nm] = nc.declare_dram_parameter(nm, shp, F32R, isOutput=False)
        import contextlib
        with contextlib.ExitStack() as ctx:
            kb.pools["const"] = ctx.enter_context(tc.tile_pool(name="const", bufs=1))
            kb.pools["wt"] = ctx.enter_context(tc.tile_pool(name="wt", bufs=2))
            kb.pools["st"] = ctx.enter_context(tc.tile_pool(name="st", bufs=2))
            kb.pools["xk"] = ctx.enter_context(tc.tile_pool(name="xk", bufs=4))
            kb.pools["coef"] = ctx.enter_context(tc.tile_pool(name="coef", bufs=4))
            kb.pools["stats"] = ctx.enter_context(tc.tile_pool(name="stats", bufs=2))
            kb.pools["psum"] = ctx.enter_context(
                tc.tile_pool(name="psum", bufs=2, space="PSUM"))

            kb.vecs_sb = kb.pools["const"].tile([128, NV], F32)
            nc.sync.dma_start(out=kb.vecs_sb, in_=vecs_d[:, :])
            kb.p2g_sb = kb.pools["const"].tile([128, 16], F32R)
            nc.sync.dma_start(out=kb.p2g_sb, in_=p2g_d[:, :])
            kb.g2p_sb = kb.pools["const"].tile([16, 128], F32R)
            nc.sync.dma_start(out=kb.g2p_sb, in_=g2p_d[:, :])
            kb.keep_w("w_clso")
            kb.keep_w("w_rlo")
            kb.zt32 = kb.pools["const"].tile([128, 1], F32)
            nc.vector.memset(kb.zt32, 0.0)
            kb.zt = kb.pools["const"].tile([128, 1], F32R)
            nc.vector.tensor_copy(kb.zt, kb.zt32)
            for nm, L in LVL:
                zero_dram_pads(kb, fbs[nm], L)

            # ---------------- FPN (C5/p5pre resident, fmaps -> DRAM) --------
            with tc.tile_pool(name="c5", bufs=2) as c5p, \
                 tc.tile_pool(name="p5pre", bufs=2) as p5prep:
                c5 = alloc_fmap(kb, c5p, L5, "c5")
                for cib in range(2):
                    nc.sync.dma_start(out=c5[cib][:, 1:L5 + 1], in_=c5_d[cib])
                p5pre = alloc_fmap(kb, p5prep, L5, "p5pre")

                w51 = kb.get_w("w_p5_1")
                b51 = kb.vcol("b_p5_1", 2)

                def cb_p5pre(cob, lt, ps):
                    nc.vector.tensor_scalar(
                        out=p5pre[cob][:, 1 + lt * LT: 1 + (lt + 1) * LT],
                        in0=ps, scalar1=b51[:, cob:cob + 1], scalar2=None,
                        op0=ALU.add)

                conv256(kb, w51, lambda cib, t, lt:
                        c5[cib][:, 1 + lt * LT: 1 + (lt + 1) * LT],
                        2, cb_p5pre, L5)

                # P4 path: p4pre = conv1x1(C4) + up2(p5pre); p4 -> DRAM
                with tc.tile_pool(name="p4pre", bufs=2) as p4prep:
                    p4pre = alloc_fmap(kb, p4prep, L4, "p4pre")
                    w41 = kb.get_w("w_p4_1")
                    b41 = kb.vcol("b_p4_1", 2)
                    for lt in range(L4 // LT):
                        xc = kb.pools["xk"].tile([128, LT + 2], F32R, tag="xk",
                                                 name="c4c")
                        nc.sync.dma_start(out=xc[:, 0:LT],
                                          in_=c4_d[:, lt * LT:(lt + 1) * LT])
                        for cob in range(2):
                            ps = kb.pools["psum"].tile([128, LT], F32, tag="ps",
                                                       bufs=3)
                            nc.tensor.matmul(ps, lhsT=w41[:, cob, :],
                                             rhs=xc[:, 0:LT],
                                             start=True, stop=True)
                            ts = kb.pools["st"].tile([128, LT], F32R, tag="ts",
                                                     bufs=3, name="ts")
                            nc.vector.tensor_scalar(out=ts, in0=ps,
                                                    scalar1=b41[:, cob:cob + 1],
                                                    scalar2=None, op0=ALU.add)
                            dst = p4pre[cob][:, 1 + lt * LT: 1 + (lt + 1) * LT]
                            src = p5pre[cob][:, 1 + lt * (LT // 2):]
                            rep = bass.AP(tensor=src.tensor, offset=src.offset,
                                          ap=[src.ap[0], [1, LT // 2], [0, 2]])
                            nc.vector.tensor_tensor(
                                out=dst.rearrange("p (a b) -> p a b", b=2),
                                in0=ts.rearrange("p (a b) -> p a b", b=2),
                                in1=rep, op=ALU.add)
                    conv256(kb, kb.get_w("w_p4_2"), rhs_resident(p4pre), 2,
                            cb_stream(kb, fbs["P4"], kb.vcol("b_p4_2", 2)), L4)

                # P5 = conv3(p5pre) -> DRAM
                conv256(kb, kb.get_w("w_p5_2"), rhs_resident(p5pre), 2,
                        cb_stream(kb, fbs["P5"], kb.vcol("b_p5_2", 2)), L5)

                # P6/P7/P8 chain (relu fmaps transient in SBUF)
                def rhs_strided(x_pair):
                    def fn(cib, t, lt):
                        sl = x_pair[cib][:, 2 * lt * LT + t:]
                        return _strided(sl, 2, LT)
                    return fn

                with tc.tile_pool(name="p6r", bufs=2) as p6rp:
                    p6r = alloc_fmap(kb, p6rp, L6, "p6r")
                    conv256(kb, kb.get_w("w_p6"), rhs_strided(c5), 2,
                            cb_stream(kb, fbs["P6"], kb.vcol("b_p6", 2), p6r), L6)
                    with tc.tile_pool(name="p7r", bufs=2) as p7rp:
                        p7r = alloc_fmap(kb, p7rp, L7, "p7r")
                        conv256(kb, kb.get_w("w_p7"), rhs_strided(p6r), 2,
                                cb_stream(kb, fbs["P7"], kb.vcol("b_p7", 2),
                                          p7r), L7)
                        conv256(kb, kb.get_w("w_p8"), rhs_strided(p7r), 2,
                                cb_stream(kb, fbs["P8"], kb.vcol("b_p8", 2)), L8)

            # ---------------- heads (conv1 streamed from DRAM fmaps) --------
            import os as _os
            _nlvl = int(_os.environ.get("KLVL", "5"))
            for nm, L in [("P8", L8), ("P7", L7), ("P6", L6),
                          ("P5", L5), ("P4", L4)][:_nlvl]:
                with tc.tile_pool(name="o_" + nm, bufs=2) as op:
                    o2c = run_head(kb, "cls", rhs_stream(kb, fbs[nm], nm + "c"),
                                   2, L, op)
                    out_convs(kb, o2c, "cls", L, out_d, OUT_OFF[nm])
                    o2r = run_head(kb, "reg", rhs_stream(kb, fbs[nm], nm + "r"),
                                   2, L, op)
                    out_convs(kb, o2r, "rl", L, out_d, OUT_OFF[nm])

    nc.finalize()
    return nc


def _dummy_params():
    """Zero-filled params with correct shapes, for building the VC column map."""
    z = lambda *s: np.zeros(s, np.float32)
    fpn_p = {"p5_1": (z(256, 256, 1), z(256)), "p5_2": (z(256, 256, 3), z(256)),
             "p4_1": (z(256, 128, 1), z(256)), "p4_2": (z(256, 256, 3), z(256)),
             "p6": (z(256, 256, 3), z(256)), "p7_2": (z(256, 256, 3), z(256)),
             "p8_2": (z(256, 256, 3), z(256))}
    cls_p = {"conv1": (z(256, 256, 3), z(256)), "gn1": (z(256), z(256)),
             "conv2": (z(256, 256, 3), z(256)), "gn2": (z(256), z(256)),
             "out": (z(2, 256, 3), z(2))}
    reg_p = {"conv1": (z(256, 256, 3), z(256)), "gn1": (z(256), z(256)),
             "conv2": (z(256, 256, 3), z(256)), "gn2": (z(256), z(256)),
             "reg": (z(2, 256, 3), z(2)), "left": (z(1, 256, 3), z(1))}
    return fpn_p, cls_p, reg_p


_NC_CACHE = {}
_LAST_EXEC_NS = {}


def kernel(C4, C5, fpn_p, cls_p, reg_p):
    C4 = np.asarray(C4, np.float32)
    C5 = np.asarray(C5, np.float32)
    assert C4.shape == (B, 128, L4) and C5.shape == (B, 256, L5)
    weights, _ = _prep_params(fpn_p, cls_p, reg_p)
    if "nc" not in _NC_CACHE:
        _NC_CACHE["nc"] = build()
    nc = _NC_CACHE["nc"]
    in_maps = []
    for b in range(B):
        m = {"C4": np.ascontiguousarray(C4[b]),
             "C5": np.ascontiguousarray(C5[b].reshape(2, 128, L5))}
        m.update(weights)
        in_maps.append(m)
    import os
    trace = bool(int(os.environ.get("BASS_KERNEL_TRACE", "0")))
    res = run_bass_kernel_spmd(nc, in_maps, core_ids=list(range(B)),
                               trace=trace)
    _LAST_EXEC_NS["ns"] = res.exec_time_ns
    cls = np.stack([r["out"][0:2, :].T for r in res.results])
    reg = np.stack([r["out"][2:4, :].T for r in res.results])
    left = np.stack([r["out"][4:5, :].T for r in res.results])
    return (np.ascontiguousarray(cls), np.ascontiguousarray(reg),
            np.ascontiguousarray(left))


# revision 28
# speedup vs baseline: 1.0594x; 1.0594x over previous
"""BeatFCOS (1D FCOS detection head) Trainium2 Bass kernel.

Strategy: data-parallel over batch (8 samples -> 8 NeuronCores, SPMD).
Each core runs the full FPN + cls/reg heads for one sample.

  - convs as f32r matmuls (PSUM fp32 accumulate), 512-wide L tiles
  - GroupNorm: bn_stats/bn_aggr per channel, conv-bias folded into GN
    coefficients, cross-partition group pooling via tiny blockdiag matmuls
  - GN apply + ReLU fused into one ScalarE activation per tile
  - P4 feature map (L=8192) spilled to DRAM and streamed for head conv1
  - outputs written as [5, 15872] per core ((cls0,cls1,reg0,reg1,left) x L),
    transposed/split on host

Self-contained: hardcodes all shapes from the problem spec.
"""
import sys
sys.path.insert(0, "/opt/trn_rl_repo")
import numpy as np

import concourse.bass as bass
import concourse.bacc as bacc
import concourse.tile as tile
from concourse import mybir
from concourse.bass_utils import run_bass_kernel_spmd

F32 = mybir.dt.float32
F32R = mybir.dt.float32r
AF = mybir.ActivationFunctionType
ALU = mybir.AluOpType

B = 8
FEAT = 256
GROUPS = 32
EPS = 1e-5
LT = 512                      # L-tile width (= one PSUM bank of fp32)
L4, L5, L6, L7, L8 = 8192, 4096, 2048, 1024, 512
LTOT = L4 + L5 + L6 + L7 + L8          # 15872
OUT_OFF = {"P4": 0, "P5": L4, "P6": L4 + L5, "P7": L4 + L5 + L6,
           "P8": L4 + L5 + L6 + L7}

# ---------------------------------------------------------------- host packing


def _lhsT_pack(w):
    """w: [Cout, Cin, K] -> [n_tiles, 128, M] ordered (cib, t, cob), lhsT[ci, co]."""
    Cout, Cin, K = w.shape
    cibs, cobs = Cin // 128, max(Cout // 128, 1)
    M = 128 if Cout >= 128 else Cout
    arr = np.ascontiguousarray(np.transpose(w, (1, 2, 0)))  # [Cin, K, Cout]
    tiles = []
    for cib in range(cibs):
        for t in range(K):
            for cob in range(cobs):
                tiles.append(arr[cib * 128:(cib + 1) * 128, t,
                                 cob * M:(cob + 1) * M])
    return np.ascontiguousarray(np.stack(tiles)).astype(np.float32)


def _col2(v):
    """[256] -> [128, 2] (col j = cob j)."""
    return np.ascontiguousarray(np.asarray(v, np.float32).reshape(2, 128).T)


def _prep_params(fpn_p, cls_p, reg_p):
    g = lambda x: np.asarray(x, np.float32)
    weights = {
        "w_p5_1": _lhsT_pack(g(fpn_p["p5_1"][0])),
        "w_p5_2": _lhsT_pack(g(fpn_p["p5_2"][0])),
        "w_p4_1": _lhsT_pack(g(fpn_p["p4_1"][0])),
        "w_p4_2": _lhsT_pack(g(fpn_p["p4_2"][0])),
        "w_p6": _lhsT_pack(g(fpn_p["p6"][0])),
        "w_p7": _lhsT_pack(g(fpn_p["p7_2"][0])),
        "w_p8": _lhsT_pack(g(fpn_p["p8_2"][0])),
        "w_cls1": _lhsT_pack(g(cls_p["conv1"][0])),
        "w_cls2": _lhsT_pack(g(cls_p["conv2"][0])),
        "w_reg1": _lhsT_pack(g(reg_p["conv1"][0])),
        "w_reg2": _lhsT_pack(g(reg_p["conv2"][0])),
        "w_clso": _lhsT_pack(g(cls_p["out"][0])),                # [6,128,2]
        "w_rlo": _lhsT_pack(np.concatenate(
            [g(reg_p["reg"][0]), g(reg_p["left"][0])], axis=0)),  # [6,128,3]
    }
    # vecs: [128, NV] fp32, column map below
    cols = []
    VC = {}

    def add2(name, arr128x2):
        VC[name] = len(cols)
        cols.append(arr128x2[:, 0])
        cols.append(arr128x2[:, 1])

    def add1(name, vec, rows):
        VC[name] = len(cols)
        c = np.zeros(128, np.float32)
        c[:rows] = vec
        cols.append(c)

    for nm, p in [("p5_1", fpn_p["p5_1"]), ("p5_2", fpn_p["p5_2"]),
                  ("p4_1", fpn_p["p4_1"]), ("p4_2", fpn_p["p4_2"]),
                  ("p6", fpn_p["p6"]), ("p7", fpn_p["p7_2"]),
                  ("p8", fpn_p["p8_2"])]:
        add2("b_" + nm, _col2(p[1]))
    for nm, p in [("cls1", cls_p["conv1"]), ("cls2", cls_p["conv2"]),
                  ("reg1", reg_p["conv1"]), ("reg2", reg_p["conv2"])]:
        add2("b_" + nm, _col2(p[1]))
    add2("g_cls1", _col2(cls_p["gn1"][0])); add2("be_cls1", _col2(cls_p["gn1"][1]))
    add2("g_cls2", _col2(cls_p["gn2"][0])); add2("be_cls2", _col2(cls_p["gn2"][1]))
    add2("g_reg1", _col2(reg_p["gn1"][0])); add2("be_reg1", _col2(reg_p["gn1"][1]))
    add2("g_reg2", _col2(reg_p["gn2"][0])); add2("be_reg2", _col2(reg_p["gn2"][1]))
    add1("eps", np.full(128, EPS, np.float32), 128)
    add1("b_clso", g(cls_p["out"][1]), 2)
    add1("b_rlo", np.concatenate([g(reg_p["reg"][1]), g(reg_p["left"][1])]), 3)
    vecs = np.ascontiguousarray(np.stack(cols, axis=1))
    VC["__ncols"] = vecs.shape[1]

    p2g = np.zeros((128, 16), np.float32)
    for gi in range(16):
        p2g[gi * 8:(gi + 1) * 8, gi] = 1.0 / 8.0
    g2p = np.zeros((16, 128), np.float32)
    for gi in range(16):
        g2p[gi, gi * 8:(gi + 1) * 8] = 1.0
    weights["vecs"] = vecs
    weights["p2g"] = p2g
    weights["g2p"] = g2p
    return weights, VC


# ---------------------------------------------------------------- device build

class KB:
    """Kernel builder state."""

    def __init__(self, nc, tc, VC):
        self.nc, self.tc, self.VC = nc, tc, VC
        self.pools = {}
        self.wdram = {}
        self.wsb_cache = {}

    def vcol(self, name, n=1, rows=128):
        c = self.VC[name]
        return self.vecs_sb[0:rows, c:c + n]

    def get_w(self, name):
        """Stream a conv-weight pack into SBUF (tag-shared double buffer)."""
        if name in self.wsb_cache:
            return self.wsb_cache[name]
        d = self.wdram[name]
        n, _, m = d.shape
        t = self.pools["wt"].tile([128, n * m], F32R, tag="wt")
        self.nc.sync.dma_start(out=t.rearrange("p (n m) -> p n m", n=n),
                               in_=d.rearrange("n p m -> p n m"))
        return t.rearrange("p (n m) -> p n m", n=n)

    def keep_w(self, name):
        """Load a weight pack into a resident (bufs=1) slot."""
        d = self.wdram[name]
        n, _, m = d.shape
        t = self.pools["const"].tile([128, n * m], F32R, tag="w_" + name)
        self.nc.sync.dma_start(out=t.rearrange("p (n m) -> p n m", n=n),
                               in_=d.rearrange("n p m -> p n m"))
        self.wsb_cache[name] = t.rearrange("p (n m) -> p n m", n=n)
        return self.wsb_cache[name]


def _strided(ap_slice, step, count):
    return bass.AP(tensor=ap_slice.tensor, offset=ap_slice.offset,
                   ap=[ap_slice.ap[0], [step, count]])


def alloc_fmap(kb, pool, L, tag, bufs=None):
    """Allocate a padded [2][128, L+2] f32r fmap, zero the pad columns."""
    nc = kb.nc
    ts = []
    for cob in range(2):
        t = pool.tile([128, L + 2], F32R, tag=tag, bufs=bufs)
        nc.vector.tensor_copy(t[:, 0:1], kb.zt32)
        nc.vector.tensor_copy(t[:, L + 1:L + 2], kb.zt32)
        ts.append(t)
    return ts


def conv256(kb, w_sb, rhs_fn, n_cib, out_cb, L, stats=None, psname="ps"):
    """Generic Cin=(n_cib*128) -> 256 conv over L outputs (pre-sliced rhs via
    rhs_fn(cib, t, lt) -> AP [128, 512]). out_cb(cob, lt, psum) emits epilogue.
    Weight pack w_sb: [128, n, M] ordered (cib, t, cob). taps inferred."""
    nc = kb.nc
    n = w_sb.shape[1]
    taps = n // (n_cib * 2)
    for lt in range(L // LT):
        for cob in range(2):
            ps = kb.pools["psum"].tile([128, LT], F32, tag=psname, bufs=4)
            k = 0
            for cib in range(n_cib):
                for t in range(taps):
                    idx = (cib * taps + t) * 2 + cob
                    nc.tensor.matmul(ps, lhsT=w_sb[:, idx, :],
                                     rhs=rhs_fn(cib, t, lt),
                                     start=(k == 0), stop=(k == n_cib * taps - 1))
                    k += 1
            out_cb(cob, lt, ps)


def emit_gn_coefs(kb, stats_pair, nch, bc2, gamma2, beta2):
    """stats_pair: 2 tiles [128, nch, 6]. Returns (scale2, bias2) [128,2] fp32."""
    nc = kb.nc
    coef = kb.pools["coef"]
    mv = coef.tile([128, 2, 2], F32, tag="mv")
    for cob in range(2):
        nc.vector.bn_aggr(out=mv[:, cob, :], in_=stats_pair[cob])
    mean_v = mv[:, :, 0]          # [128, 2] strided
    var_v = mv[:, :, 1]
    pm = coef.tile([128, 4], F32R, tag="pm")
    s1 = coef.tile([128, 2], F32, tag="s1")
    s2 = coef.tile([128, 2], F32, tag="s2")
    s3 = coef.tile([128, 2], F32, tag="s3")
    # m1b = mean + bc ; m2b = E[(x+b)^2] = var + m1b^2
    nc.vector.tensor_tensor(out=pm[:, 0:2], in0=mean_v, in1=bc2, op=ALU.add)
    nc.vector.tensor_tensor(out=s1, in0=pm[:, 0:2], in1=pm[:, 0:2], op=ALU.mult)
    nc.vector.tensor_tensor(out=pm[:, 2:4], in0=s1, in1=var_v, op=ALU.add)
    gps = kb.pools["psum"].tile([16, 4], F32, tag="gps")
    nc.tensor.matmul(gps, lhsT=kb.p2g_sb, rhs=pm, start=True, stop=True)
    gsb = coef.tile([16, 4], F32R, tag="gsb")
    nc.vector.tensor_copy(gsb, gps)
    bps = kb.pools["psum"].tile([128, 4], F32, tag="bps")
    nc.tensor.matmul(bps, lhsT=kb.g2p_sb, rhs=gsb, start=True, stop=True)
    bcp = coef.tile([128, 4], F32, tag="bcp")
    nc.vector.tensor_copy(bcp, bps)
    # var_g = m2g - mean_g^2 ; rstd = 1/sqrt(var+eps)
    nc.vector.tensor_tensor(out=s1, in0=bcp[:, 0:2], in1=bcp[:, 0:2], op=ALU.mult)
    nc.vector.tensor_tensor(out=s1, in0=bcp[:, 2:4], in1=s1, op=ALU.subtract)
    nc.scalar.activation(out=s2, in_=s1, func=AF.Sqrt,
                         bias=kb.vcol("eps"), scale=1.0)
    nc.vector.reciprocal(out=s2, in_=s2)                                 # rstd
    scale2 = coef.tile([128, 2], F32, tag="scale2")
    bias2 = coef.tile([128, 2], F32, tag="bias2")
    nc.vector.tensor_tensor(out=scale2, in0=gamma2, in1=s2, op=ALU.mult)
    nc.vector.tensor_tensor(out=s3, in0=bc2, in1=bcp[:, 0:2], op=ALU.subtract)
    nc.vector.tensor_tensor(out=s3, in0=s3, in1=scale2, op=ALU.mult)
    nc.vector.tensor_tensor(out=bias2, in0=s3, in1=beta2, op=ALU.add)
    return scale2, bias2


def apply_tile(kb, o_pair, scale2, bias2, lt):
    """GN-apply+ReLU for one L-chunk, both channel blocks (ACT + DVE/Pool)."""
    nc = kb.nc
    for cob in range(2):
        sl = o_pair[cob][:, 1 + lt * LT: 1 + (lt + 1) * LT]
        if cob == 0:
            nc.scalar.activation(out=sl, in_=sl, func=AF.Relu,
                                 bias=bias2[:, cob:cob + 1],
                                 scale=scale2[:, cob:cob + 1])
        else:
            eng = nc.gpsimd if lt % 2 == 0 else nc.vector
            eng.tensor_scalar(out=sl, in0=sl,
                              scalar1=scale2[:, cob:cob + 1],
                              scalar2=bias2[:, cob:cob + 1],
                              op0=ALU.mult, op1=ALU.add)
            eng.tensor_scalar_max(out=sl, in0=sl, scalar1=0.0)


def staged_conv_after_gn(kb, w_name, o1, s2, b2, L, o_pool, otag, stats_tag,
                         obufs=4):
    """Apply GN to o1 tile-by-tile, staggered one tile ahead of conv2."""
    nc = kb.nc
    o2 = alloc_fmap(kb, o_pool, L, otag, bufs=obufs)
    nch = L // LT
    stats = [kb.pools["stats"].tile([128, nch, 6], F32, tag=stats_tag + str(c),
                                    name=f"stats_{stats_tag}{c}")
             for c in range(2)]
    w_sb = kb.get_w(w_name)
    apply_tile(kb, o1, s2, b2, 0)
    for lt in range(nch):
        if lt + 1 < nch:
            apply_tile(kb, o1, s2, b2, lt + 1)
        for cob in range(2):
            ps = kb.pools["psum"].tile([128, LT], F32, tag="ps", bufs=4)
            k = 0
            for cib in range(2):
                for t in range(3):
                    idx = (cib * 3 + t) * 2 + cob
                    nc.tensor.matmul(
                        ps, lhsT=w_sb[:, idx, :],
                        rhs=o1[cib][:, lt * LT + t: lt * LT + t + LT],
                        start=(k == 0), stop=(k == 5))
                    k += 1
            sl = o2[cob][:, 1 + lt * LT: 1 + (lt + 1) * LT]
            if cob == 0:
                nc.vector.tensor_copy(sl, ps)
            else:
                nc.scalar.copy(sl, ps)
            nc.vector.bn_stats(out=stats[cob][:, lt, :], in_=sl)
    return o2, stats


def out_convs_after_gn(kb, o2, s2, b2, which, L, out_d, off):
    """Apply GN to o2 staggered one tile ahead of the final out-convs."""
    nc = kb.nc
    nch = L // LT
    apply_tile(kb, o2, s2, b2, 0)
    for lt in range(nch):
        if lt + 1 < nch:
            apply_tile(kb, o2, s2, b2, lt + 1)
        if which == "cls":
            cps = kb.pools["psum"].tile([2, LT], F32, tag="ps", bufs=4)
            for k, (cib, t) in enumerate(
                    [(c, t) for c in range(2) for t in range(3)]):
                nc.tensor.matmul(cps, lhsT=kb.wsb_cache["w_clso"][:, cib * 3 + t, :],
                                 rhs=o2[cib][:, lt * LT + t: lt * LT + t + LT],
                                 start=(k == 0), stop=(k == 5))
            cst = kb.pools["st"].tile([2, LT], F32, tag="cst", bufs=2)
            nc.scalar.activation(out=cst, in_=cps, func=AF.Sigmoid,
                                 bias=kb.vcol("b_clso", rows=2), scale=1.0)
            nc.sync.dma_start(out=out_d[0:2, off + lt * LT: off + (lt + 1) * LT],
                              in_=cst)
        else:
            rps = kb.pools["psum"].tile([3, LT], F32, tag="ps", bufs=4)
            for k, (cib, t) in enumerate(
                    [(c, t) for c in range(2) for t in range(3)]):
                nc.tensor.matmul(rps, lhsT=kb.wsb_cache["w_rlo"][:, cib * 3 + t, :],
                                 rhs=o2[cib][:, lt * LT + t: lt * LT + t + LT],
                                 start=(k == 0), stop=(k == 5))
            rst = kb.pools["st"].tile([3, LT], F32, tag="rst", bufs=2)
            nc.scalar.activation(out=rst, in_=rps, func=AF.Sigmoid,
                                 bias=kb.vcol("b_rlo", rows=3), scale=1.0)
            nc.vector.tensor_scalar(out=rst[0:2, :], in0=rps[0:2, :],
                                    scalar1=kb.vcol("b_rlo", rows=2), scalar2=None,
                                    op0=ALU.add)
            nc.sync.dma_start(out=out_d[2:5, off + lt * LT: off + (lt + 1) * LT],
                              in_=rst)


def gn_apply(kb, o_pair, L, scale2, bias2):
    nc = kb.nc
    for cob in range(2):
        for lt in range(L // LT):
            sl = o_pair[cob][:, 1 + lt * LT: 1 + (lt + 1) * LT]
            if cob == 0:
                nc.scalar.activation(out=sl, in_=sl, func=AF.Relu,
                                     bias=bias2[:, cob:cob + 1],
                                     scale=scale2[:, cob:cob + 1])
            else:
                eng = nc.gpsimd if lt % 2 == 0 else nc.vector
                eng.tensor_scalar(out=sl, in0=sl,
                                  scalar1=scale2[:, cob:cob + 1],
                                  scalar2=bias2[:, cob:cob + 1],
                                  op0=ALU.mult, op1=ALU.add)
                eng.tensor_scalar_max(out=sl, in0=sl, scalar1=0.0)


def head_conv(kb, w_name, rhs_fn, n_cib, L, o_pool, otag, stats_tag, obufs=None):
    """conv + copy-out (f32r) + bn_stats. Returns (o_pair, stats_pair)."""
    nc = kb.nc
    o_pair = alloc_fmap(kb, o_pool, L, otag, bufs=obufs)
    nch = L // LT
    stats = [kb.pools["stats"].tile([128, nch, 6], F32, tag=stats_tag + str(c),
                                    name=f"stats_{stats_tag}{c}")
             for c in range(2)]
    w_sb = kb.get_w(w_name)

    def cb(cob, lt, ps):
        sl = o_pair[cob][:, 1 + lt * LT: 1 + (lt + 1) * LT]
        if cob == 0:
            nc.vector.tensor_copy(sl, ps)
        else:
            nc.scalar.copy(sl, ps)
        nc.vector.bn_stats(out=stats[cob][:, lt, :], in_=sl)

    conv256(kb, w_sb, rhs_fn, n_cib, cb, L)
    return o_pair, stats


def rhs_resident(x_pair):
    """rhs provider for a resident padded fmap pair."""
    def fn(cib, t, lt):
        return x_pair[cib][:, lt * LT + t: lt * LT + t + LT]
    return fn


def out_convs(kb, o2n, which, L, out_d, off):
    """Final 1x3 convs for one head + epilogue + DMA to out rows."""
    nc = kb.nc
    for lt in range(L // LT):
        if which == "cls":
            cps = kb.pools["psum"].tile([2, LT], F32, tag="ps", bufs=4)
            for k, (cib, t) in enumerate(
                    [(c, t) for c in range(2) for t in range(3)]):
                nc.tensor.matmul(cps, lhsT=kb.wsb_cache["w_clso"][:, cib * 3 + t, :],
                                 rhs=o2n[cib][:, lt * LT + t: lt * LT + t + LT],
                                 start=(k == 0), stop=(k == 5))
            cst = kb.pools["st"].tile([2, LT], F32, tag="cst", bufs=2)
            nc.scalar.activation(out=cst, in_=cps, func=AF.Sigmoid,
                                 bias=kb.vcol("b_clso", rows=2), scale=1.0)
            nc.sync.dma_start(out=out_d[0:2, off + lt * LT: off + (lt + 1) * LT],
                              in_=cst)
        else:
            rps = kb.pools["psum"].tile([3, LT], F32, tag="ps", bufs=4)
            for k, (cib, t) in enumerate(
                    [(c, t) for c in range(2) for t in range(3)]):
                nc.tensor.matmul(rps, lhsT=kb.wsb_cache["w_rlo"][:, cib * 3 + t, :],
                                 rhs=o2n[cib][:, lt * LT + t: lt * LT + t + LT],
                                 start=(k == 0), stop=(k == 5))
            rst = kb.pools["st"].tile([3, LT], F32, tag="rst", bufs=2)
            # sigmoid all 3 rows (only row 2 = left is kept) ...
            nc.scalar.activation(out=rst, in_=rps, func=AF.Sigmoid,
                                 bias=kb.vcol("b_rlo", rows=3), scale=1.0)
            # ... then overwrite rows 0:2 with the plain reg bias-add
            nc.vector.tensor_scalar(out=rst[0:2, :], in0=rps[0:2, :],
                                    scalar1=kb.vcol("b_rlo", rows=2), scalar2=None,
                                    op0=ALU.add)
            nc.sync.dma_start(out=out_d[2:5, off + lt * LT: off + (lt + 1) * LT],
                              in_=rst)


def gn_for(kb, kind, layer, stats_pair, L):
    return emit_gn_coefs(kb, stats_pair, L // LT, kb.vcol(f"b_{kind}{layer}", 2),
                         kb.vcol(f"g_{kind}{layer}", 2),
                         kb.vcol(f"be_{kind}{layer}", 2))


def head_conv1_fused(kb, rhs_fn, L, o_pool):
    """conv1 of BOTH heads in one L-loop, sharing streamed rhs windows."""
    nc = kb.nc
    o1c = alloc_fmap(kb, o_pool, L, "o1", bufs=4)
    o1r = alloc_fmap(kb, o_pool, L, "o1", bufs=4)
    nch = L // LT
    sts = {}
    for kind in ("cls", "reg"):
        sts[kind] = [kb.pools["stats"].tile([128, nch, 6], F32,
                                            tag=f"s1{kind}{c}",
                                            name=f"stats_s1{kind}{c}")
                     for c in range(2)]
    wc = kb.get_w("w_cls1")
    wr = kb.get_w("w_reg1")

    def emit(kind, w_sb, o1, lt):
        for cob in range(2):
            ps = kb.pools["psum"].tile([128, LT], F32, tag="ps", bufs=4,
                                       name="ps")
            k = 0
            for cib in range(2):
                for t in range(3):
                    idx = (cib * 3 + t) * 2 + cob
                    nc.tensor.matmul(ps, lhsT=w_sb[:, idx, :],
                                     rhs=rhs_fn(cib, t, lt),
                                     start=(k == 0), stop=(k == 5))
                    k += 1
            sl = o1[cob][:, 1 + lt * LT: 1 + (lt + 1) * LT]
            if cob == 0:
                nc.vector.tensor_copy(sl, ps)
            else:
                nc.scalar.copy(sl, ps)
            nc.vector.bn_stats(out=sts[kind][cob][:, lt, :], in_=sl)

    # reg lags cls by one tile; the final reg tile is emitted by the caller
    # AFTER the GN1-cls coef chain so PE work covers it.
    for lt in range(nch):
        emit("cls", wc, o1c, lt)
        if lt > 0:
            emit("reg", wr, o1r, lt - 1)
    return o1c, sts["cls"], o1r, sts["reg"], (
        lambda: emit("reg", wr, o1r, nch - 1))


def run_heads_bridged(kb, L, op, rhs_maker, out_d, off, next_prefetch=None):
    """Both heads with interleaved stage emission so PE bridges GN stalls."""
    o1c, st1c, o1r, st1r, emit_last_reg = head_conv1_fused(
        kb, rhs_maker(), L, op)
    s1, b1 = gn_for(kb, "cls", 1, st1c, L)
    emit_last_reg()
    o2c, st2c = staged_conv_after_gn(kb, "w_cls2", o1c, s1, b1, L, op, "o2",
                                     "s2cls")
    s1r, b1r = gn_for(kb, "reg", 1, st1r, L)
    s2, b2 = gn_for(kb, "cls", 2, st2c, L)
    o2r, st2r = staged_conv_after_gn(kb, "w_reg2", o1r, s1r, b1r, L, op, "o2",
                                     "s2reg")
    s2r, b2r = gn_for(kb, "reg", 2, st2r, L)
    if next_prefetch is not None:
        next_prefetch()
    out_convs_after_gn(kb, o2c, s2, b2, "cls", L, out_d, off)
    out_convs_after_gn(kb, o2r, s2r, b2r, "rl", L, out_d, off)


def run_head(kb, kind, rhs1_fn, n_cib, L, o_pool):
    """One head (cls or reg): conv1 -> GN1 -> conv2 -> GN2. Returns o2n pair."""
    o1, st1 = head_conv(kb, f"w_{kind}1", rhs1_fn, n_cib, L, o_pool,
                        "o1", f"s1{kind}")
    s1c, b1c = emit_gn_coefs(kb, st1, L // LT, kb.vcol(f"b_{kind}1", 2),
                             kb.vcol(f"g_{kind}1", 2), kb.vcol(f"be_{kind}1", 2))
    gn_apply(kb, o1, L, s1c, b1c)
    o2, st2 = head_conv(kb, f"w_{kind}2", rhs_resident(o1), 2, L, o_pool,
                        "o2", f"s2{kind}")
    s2c, b2c = emit_gn_coefs(kb, st2, L // LT, kb.vcol(f"b_{kind}2", 2),
                             kb.vcol(f"g_{kind}2", 2), kb.vcol(f"be_{kind}2", 2))
    gn_apply(kb, o2, L, s2c, b2c)
    return o2


def rhs_stream(kb, fb, hint):
    """conv1 rhs provider streaming [128, LT+2] windows from a DRAM fmap."""
    nc = kb.nc
    cache = {}

    def fn(cib, t, lt):
        key = (cib, lt)
        if key not in cache:
            xk = kb.pools["xk"].tile([128, LT + 2], F32R, tag="xk",
                                     name=f"xk_{hint}")
            h = LT // 2 + 1
            nc.sync.dma_start(out=xk[:, 0:h],
                              in_=fb[cib][:, lt * LT: lt * LT + h])
            nc.sync.dma_start(out=xk[:, h:LT + 2],
                              in_=fb[cib][:, lt * LT + h: lt * LT + LT + 2])
            cache[key] = xk
        return cache[key][:, t: t + LT]

    fn.prefetch = lambda: (fn(0, 0, 0), fn(1, 0, 0))
    return fn


def zero_dram_pads(kb, fb, L):
    nc = kb.nc
    for cib in range(2):
        nc.sync.dma_start(out=fb[cib][:, 0:1], in_=kb.zt)
        nc.sync.dma_start(out=fb[cib][:, L + 1:L + 2], in_=kb.zt)


def cb_stream(kb, fb, bcol, relu_pair=None):
    """Epilogue: bias-add -> DMA to DRAM fmap; optionally also ReLU -> SBUF pair."""
    nc = kb.nc

    def cb(cob, lt, ps):
        ts = kb.pools["st"].tile([128, LT], F32R, tag="ts", bufs=3, name="ts")
        nc.any.tensor_scalar(out=ts, in0=ps, scalar1=bcol[:, cob:cob + 1],
                             scalar2=None, op0=ALU.add)
        nc.sync.dma_start(out=fb[cob][:, 1 + lt * LT: 1 + (lt + 1) * LT], in_=ts)
        if relu_pair is not None:
            nc.scalar.activation(
                out=relu_pair[cob][:, 1 + lt * LT: 1 + (lt + 1) * LT],
                in_=ps, func=AF.Relu, bias=bcol[:, cob:cob + 1], scale=1.0)
    return cb


def build():
    nc = bacc.Bacc()
    c4_d = nc.declare_dram_parameter("C4", [128, L4], F32R, isOutput=False)
    c5_d = nc.declare_dram_parameter("C5", [2, 128, L5], F32R, isOutput=False)
    wshapes = {
        "w_p5_1": [4, 128, 128], "w_p5_2": [12, 128, 128],
        "w_p4_1": [2, 128, 128], "w_p4_2": [12, 128, 128],
        "w_p6": [12, 128, 128], "w_p7": [12, 128, 128], "w_p8": [12, 128, 128],
        "w_cls1": [12, 128, 128], "w_cls2": [12, 128, 128],
        "w_reg1": [12, 128, 128], "w_reg2": [12, 128, 128],
        "w_clso": [6, 128, 2], "w_rlo": [6, 128, 3],
    }
    _, VC = _prep_params(*_dummy_params())
    NV = VC.pop("__ncols")
    vecs_d = nc.declare_dram_parameter("vecs", [128, NV], F32, isOutput=False)
    p2g_d = nc.declare_dram_parameter("p2g", [128, 16], F32R, isOutput=False)
    g2p_d = nc.declare_dram_parameter("g2p", [16, 128], F32R, isOutput=False)
    out_d = nc.declare_dram_parameter("out", [5, LTOT], F32, isOutput=True)
    LVL = [("P4", L4), ("P5", L5), ("P6", L6), ("P7", L7), ("P8", L8)]
    fbs = {nm: nc.dram_tensor(f"fb_{nm}", [2, 128, L + 2], F32R)
           for nm, L in LVL}

    with tile.TileContext(nc) as tc:
        kb = KB(nc, tc, VC)
        for nm, shp in wshapes.items():
            kb.wdram[nm] = nc.declare_dram_parameter(nm, shp, F32R, isOutput=False)
        import contextlib
        with contextlib.ExitStack() as ctx:
            kb.pools["const"] = ctx.enter_context(tc.tile_pool(name="const", bufs=1))
            kb.pools["wt"] = ctx.enter_context(tc.tile_pool(name="wt", bufs=3))
            kb.pools["st"] = ctx.enter_context(tc.tile_pool(name="st", bufs=2))
            kb.pools["xk"] = ctx.enter_context(tc.tile_pool(name="xk", bufs=10))
            kb.pools["coef"] = ctx.enter_context(tc.tile_pool(name="coef", bufs=4))
            kb.pools["stats"] = ctx.enter_context(tc.tile_pool(name="stats", bufs=2))
            kb.pools["psum"] = ctx.enter_context(
                tc.tile_pool(name="psum", bufs=2, space="PSUM"))

            kb.vecs_sb = kb.pools["const"].tile([128, NV], F32)
            nc.sync.dma_start(out=kb.vecs_sb, in_=vecs_d[:, :])
            kb.p2g_sb = kb.pools["const"].tile([128, 16], F32R)
            nc.sync.dma_start(out=kb.p2g_sb, in_=p2g_d[:, :])
            kb.g2p_sb = kb.pools["const"].tile([16, 128], F32R)
            nc.sync.dma_start(out=kb.g2p_sb, in_=g2p_d[:, :])
            kb.keep_w("w_clso")
            kb.keep_w("w_rlo")
            kb.zt32 = kb.pools["const"].tile([128, 1], F32)
            nc.vector.memset(kb.zt32, 0.0)
            kb.zt = kb.pools["const"].tile([128, 1], F32R)
            nc.vector.tensor_copy(kb.zt, kb.zt32)
            for nm, L in LVL:
                zero_dram_pads(kb, fbs[nm], L)

            # ---------------- FPN (C5/p5pre resident, fmaps -> DRAM) --------
            with tc.tile_pool(name="c5", bufs=2) as c5p, \
                 tc.tile_pool(name="p5pre", bufs=2) as p5prep:
                c5 = alloc_fmap(kb, c5p, L5, "c5")
                for cib in range(2):
                    for lt in range(L5 // LT):
                        nc.sync.dma_start(
                            out=c5[cib][:, 1 + lt * LT: 1 + (lt + 1) * LT],
                            in_=c5_d[cib][:, lt * LT:(lt + 1) * LT])
                p5pre = alloc_fmap(kb, p5prep, L5, "p5pre")

                w51 = kb.get_w("w_p5_1")
                b51 = kb.vcol("b_p5_1", 2)

                def cb_p5pre(cob, lt, ps):
                    nc.vector.tensor_scalar(
                        out=p5pre[cob][:, 1 + lt * LT: 1 + (lt + 1) * LT],
                        in0=ps, scalar1=b51[:, cob:cob + 1], scalar2=None,
                        op0=ALU.add)

                conv256(kb, w51, lambda cib, t, lt:
                        c5[cib][:, 1 + lt * LT: 1 + (lt + 1) * LT],
                        2, cb_p5pre, L5)

                # P5 = conv3(p5pre) -> DRAM
                conv256(kb, kb.get_w("w_p5_2"), rhs_resident(p5pre), 2,
                        cb_stream(kb, fbs["P5"], kb.vcol("b_p5_2", 2)), L5)

                # P6/P7/P8 chain (relu fmaps transient in SBUF)
                def rhs_strided(x_pair):
                    def fn(cib, t, lt):
                        sl = x_pair[cib][:, 2 * lt * LT + t:]
                        return _strided(sl, 2, LT)
                    return fn

                with tc.tile_pool(name="p6r", bufs=2) as p6rp:
                    p6r = alloc_fmap(kb, p6rp, L6, "p6r")
                    conv256(kb, kb.get_w("w_p6"), rhs_strided(c5), 2,
                            cb_stream(kb, fbs["P6"], kb.vcol("b_p6", 2), p6r), L6)
                    with tc.tile_pool(name="p7r", bufs=2) as p7rp:
                        p7r = alloc_fmap(kb, p7rp, L7, "p7r")
                        conv256(kb, kb.get_w("w_p7"), rhs_strided(p6r), 2,
                                cb_stream(kb, fbs["P7"], kb.vcol("b_p7", 2),
                                          p7r), L7)
                        conv256(kb, kb.get_w("w_p8"), rhs_strided(p7r), 2,
                                cb_stream(kb, fbs["P8"], kb.vcol("b_p8", 2)), L8)

                # P4 path: p4pre = conv1x1(C4) + up2(p5pre); p4 -> DRAM
                with tc.tile_pool(name="p4pre", bufs=2) as p4prep:
                    p4pre = alloc_fmap(kb, p4prep, L4, "p4pre")
                    w41 = kb.get_w("w_p4_1")
                    b41 = kb.vcol("b_p4_1", 2)
                    for lt in range(L4 // LT):
                        xc = kb.pools["xk"].tile([128, LT + 2], F32R, tag="xk",
                                                 name="c4c")
                        nc.sync.dma_start(out=xc[:, 0:LT // 2],
                                          in_=c4_d[:, lt * LT: lt * LT + LT // 2])
                        nc.sync.dma_start(out=xc[:, LT // 2:LT],
                                          in_=c4_d[:, lt * LT + LT // 2:(lt + 1) * LT])
                        for cob in range(2):
                            ps = kb.pools["psum"].tile([128, LT], F32, tag="ps",
                                                       bufs=4)
                            nc.tensor.matmul(ps, lhsT=w41[:, cob, :],
                                             rhs=xc[:, 0:LT],
                                             start=True, stop=True)
                            ts = kb.pools["st"].tile([128, LT], F32R, tag="ts",
                                                     bufs=3, name="ts")
                            nc.vector.tensor_scalar(out=ts, in0=ps,
                                                    scalar1=b41[:, cob:cob + 1],
                                                    scalar2=None, op0=ALU.add)
                            dst = p4pre[cob][:, 1 + lt * LT: 1 + (lt + 1) * LT]
                            src = p5pre[cob][:, 1 + lt * (LT // 2):]
                            rep = bass.AP(tensor=src.tensor, offset=src.offset,
                                          ap=[src.ap[0], [1, LT // 2], [0, 2]])
                            nc.gpsimd.tensor_tensor(
                                out=dst.rearrange("p (a b) -> p a b", b=2),
                                in0=ts.rearrange("p (a b) -> p a b", b=2),
                                in1=rep, op=ALU.add)
                    conv256(kb, kb.get_w("w_p4_2"), rhs_resident(p4pre), 2,
                            cb_stream(kb, fbs["P4"], kb.vcol("b_p4_2", 2)), L4)

            # ---------------- heads (conv1 streamed from DRAM fmaps) --------
            import os as _os
            _nlvl = int(_os.environ.get("KLVL", "5"))
            lvls = [("P8", L8), ("P7", L7), ("P6", L6),
                    ("P5", L5), ("P4", L4)][:_nlvl]
            for li, (nm, L) in enumerate(lvls):
                nxt = lvls[li + 1][0] if li + 1 < len(lvls) else None
                npf = ((lambda nxt=nxt: rhs_stream(kb, fbs[nxt],
                                                   nxt + "pf").prefetch())
                       if nxt is not None and nxt != "P4" else None)
                with tc.tile_pool(name="o_" + nm, bufs=2) as op:
                    if nm != "P4":
                        run_heads_bridged(
                            kb, L, op,
                            lambda nm=nm: rhs_stream(kb, fbs[nm], nm),
                            out_d, OUT_OFF[nm], next_prefetch=None)
                    else:
                        rhs_r = rhs_stream(kb, fbs[nm], nm + "r")
                        o1c, st1c = head_conv(
                            kb, "w_cls1", rhs_stream(kb, fbs[nm], nm + "c"),
                            2, L, op, "o1", "s1cls")
                        s1, b1 = gn_for(kb, "cls", 1, st1c, L)
                        o2c, st2c = staged_conv_after_gn(
                            kb, "w_cls2", o1c, s1, b1, L, op, "o2", "s2cls",
                            obufs=2)
                        s2, b2 = gn_for(kb, "cls", 2, st2c, L)
                        rhs_r.prefetch()
                        out_convs_after_gn(kb, o2c, s2, b2, "cls", L, out_d,
                                           OUT_OFF[nm])
                        o1r, st1r = head_conv(kb, "w_reg1", rhs_r, 2, L, op,
                                              "o1", "s1reg")
                        s1r, b1r = gn_for(kb, "reg", 1, st1r, L)
                        o2r, st2r = staged_conv_after_gn(
                            kb, "w_reg2", o1r, s1r, b1r, L, op, "o2", "s2reg",
                            obufs=2)
                        s2r, b2r = gn_for(kb, "reg", 2, st2r, L)
                        out_convs_after_gn(kb, o2r, s2r, b2r, "rl", L, out_d,
                                           OUT_OFF[nm])

    nc.finalize()
    return nc


def _dummy_params():
    """Zero-filled params with correct shapes, for building the VC column map."""
    z = lambda *s: np.zeros(s, np.float32)
    fpn_p = {"p5_1": (z(256, 256, 1), z(256)), "p5_2": (z(256, 256, 3), z(256)),
             "p4_1": (z(256, 128, 1), z(256)), "p4_2": (z(256, 256, 3), z(256)),
             "p6": (z(256, 256, 3), z(256)), "p7_2": (z(256, 256, 3), z(256)),
             "p8_2": (z(256, 256, 3), z(256))}
    cls_p = {"conv1": (z(256, 256, 3), z(256)), "gn1": (z(256), z(256)),
             "conv2": (z(256, 256, 3), z(256)), "gn2": (z(256), z(256)),
             "out": (z(2, 256, 3), z(2))}
    reg_p = {"conv1": (z(256, 256, 3), z(256)), "gn1": (z(256), z(256)),
             "conv2": (z(256, 256, 3), z(256)), "gn2": (z(256), z(256)),
             "reg": (z(2, 256, 3), z(2)), "left": (z(1, 256, 3), z(1))}
    return fpn_p, cls_p, reg_p


_NC_CACHE = {}
_LAST_EXEC_NS = {}


def kernel(C4, C5, fpn_p, cls_p, reg_p):
    C4 = np.asarray(C4, np.float32)
    C5 = np.asarray(C5, np.float32)
    assert C4.shape == (B, 128, L4) and C5.shape == (B, 256, L5)
    weights, _ = _prep_params(fpn_p, cls_p, reg_p)
    if "nc" not in _NC_CACHE:
        _NC_CACHE["nc"] = build()
    nc = _NC_CACHE["nc"]
    in_maps = []
    for b in range(B):
        m = {"C4": np.ascontiguousarray(C4[b]),
             "C5": np.ascontiguousarray(C5[b].reshape(2, 128, L5))}
        m.update(weights)
        in_maps.append(m)
    import os
    trace = bool(int(os.environ.get("BASS_KERNEL_TRACE", "0")))
    res = run_bass_kernel_spmd(nc, in_maps, core_ids=list(range(B)),
                               trace=trace)
    _LAST_EXEC_NS["ns"] = res.exec_time_ns
    cls = np.stack([r["out"][0:2, :].T for r in res.results])
    reg = np.stack([r["out"][2:4, :].T for r in res.results])
    left = np.stack([r["out"][4:5, :].T for r in res.results])
    return (np.ascontiguousarray(cls), np.ascontiguousarray(reg),
            np.ascontiguousarray(left))
